# revision 1
# baseline (speedup 1.0000x reference)
"""Single-head causal attention on 8 TRN2 NeuronCores.

Problem: x [8, 2048, 1024] f32, Wq/Wk/Wv [1024, 64] f32.
  q = x @ Wq ; k = x @ Wk ; v = x @ Wv        (per batch)
  out = softmax(causal(q k^T / 8)) @ v        [8, 2048, 64]

Sharding: data-parallel over batch -- core i handles batch element i.
No collectives needed.

Per-core kernel (bf16 compute, f32 accumulate):
  1. x streams in per 512-token chunk ([128, 4, 1024] f32 DMA), is cast
     to bf16 (DVE/GPSIMD), and transposed to x^T [128 d-part, dc, t]
     via the DMA-xbar (chunks 0-2) or the PE (last chunk, whose xbar
     slot would otherwise straggle behind the loads).
  2. Projections per chunk: lhsT = packed [Wq|Wk] per 128-d-chunk
     accumulates Q^T,K^T [64, 512] in one PSUM tile; lhsT = Wv gives
     V^T [64, 512]. V^T is xbar-transposed to V [t-part, 4, 64] and
     augmented with a ones column (softmax denominator for free).
  3. Scores are computed in the transposed orientation
     S^T[tk, tq] = K^T_slice.T @ Q^T -- both operands already have h on
     partitions, so no per-tile P transposes are needed anywhere.
  4. exp on ACT (scale=1/8, no max-subtraction: scores are O(1));
     causal diagonal handled by a multiplicative 0/1 upper-triangular
     bf16 mask. q-chunks 0-1 form one 1024-wide attention block
     (halves the exp op count); the S matmul for k-tile ki+1 is issued
     before the PV matmul of ki so the PE never stalls on the exp
     round-trip.
  5. PV: out_aug^T[65, tq] += V_aug[ki].T @ P^T accumulated over
     k-tiles in PSUM; row 64 accumulates the softmax denominators.
  6. PE-transpose out_aug^T back to [tq, 65] (f32), scale rows by the
     reciprocal denominator, one batched DMA store per chunk.

Scheduling notes: DMA instruction count is minimized (each costs
~0.6us of serialized HWDGE issue plus ~1.2us sequencer time); loads,
transposes and stores are split across the SP and GPSIMD queues so no
in-order sequencer head-of-line blocks another chunk's work; xbar
transpose outputs must be 32-byte aligned in SBUF (v_aug k-tile stride
padded 65->80) -- misaligned outputs corrupt silently.

The chunks are processed in the order 0, 1, 3, 2 (x loads in the same
order): chunk 3's attention block needs only its own projection at its
head, so its exp stream starts the moment block A's drains; its k-tiles
run with the diagonal group hoisted before 8-11 so the late-loaded
chunk 2's K/V never stall the ACT engine, and chunk 2's projections/
cast/transpose are injected into the PE's idle slots mid-block. (PSUM
accumulation is order-independent, so each block's k-tile sequence and
the injection positions are schedule-tuned against the cost model.)
The V projections are deferred past each block's first score matmuls
(V_aug is only read by the block's last PV matmuls), and the previous
block's output stage (PE transpose + rescale) is replayed mid-way
through the next block where exp widths shrink.
"""

import numpy as np

import concourse.bass as bass
import concourse.tile as tile
from concourse import bacc, mybir
from concourse.bass_utils import run_bass_kernel_spmd

B, T, D, H = 8, 2048, 1024, 64
P = 128            # partitions / tile edge
ND = D // P        # 8 d-chunks
NT = T // P        # 16 token tiles
CW = 512           # chunk width (1 PSUM bank of f32)
NC = T // CW       # 4 chunks
KPC = CW // P      # 4 k-tiles per chunk

FP32 = mybir.dt.float32
BF16 = mybir.dt.bfloat16

_compiled = None
DEBUG_DUMP = False


def _build():
    nc = bacc.Bacc("TRN2", target_bir_lowering=False, debug=False, num_devices=8)

    x_d = nc.dram_tensor("x", [T, D], FP32, kind="ExternalInput").ap()
    wq_d = nc.dram_tensor("Wq", [D, H], FP32, kind="ExternalInput").ap()
    wk_d = nc.dram_tensor("Wk", [D, H], FP32, kind="ExternalInput").ap()
    wv_d = nc.dram_tensor("Wv", [D, H], FP32, kind="ExternalInput").ap()
    out_d = nc.dram_tensor("out", [T, H], FP32, kind="ExternalOutput").ap()
    dbg = {}
    if DEBUG_DUMP:
        dbg["xt0"] = nc.dram_tensor("xt0", [P, ND, CW], FP32,
                                    kind="ExternalOutput").ap()
        dbg["vaug0"] = nc.dram_tensor("vaug0", [P, KPC, H + 1], FP32,
                                      kind="ExternalOutput").ap()
        dbg["qt0"] = nc.dram_tensor("qt0", [H, CW], FP32,
                                    kind="ExternalOutput").ap()
        dbg["kt0"] = nc.dram_tensor("kt0", [H, CW], FP32,
                                    kind="ExternalOutput").ap()

    with tile.TileContext(nc) as tc:
        _kernel(tc, out_d, x_d, wq_d, wk_d, wv_d, dbg)

    nc.compile()
    return nc


def _kernel(tc, out_d, x_d, wq_d, wk_d, wv_d, dbg=None):
    nc = tc.nc
    from contextlib import ExitStack

    ctx = ExitStack()
    with ctx:
        const = ctx.enter_context(tc.tile_pool(name="const", bufs=1))
        wstage = ctx.enter_context(tc.tile_pool(name="wstage", bufs=2))
        xload = ctx.enter_context(tc.tile_pool(name="xload", bufs=4))
        xbf = ctx.enter_context(tc.tile_pool(name="xbf", bufs=8))
        xtp = ctx.enter_context(tc.tile_pool(name="xtp", bufs=1))
        qkv = ctx.enter_context(tc.tile_pool(name="qkv", bufs=1))
        vsb = ctx.enter_context(tc.tile_pool(name="vsb", bufs=1))
        ptp = ctx.enter_context(tc.tile_pool(name="ptp", bufs=7))
        otp = ctx.enter_context(tc.tile_pool(name="otp", bufs=2))
        osb = ctx.enter_context(tc.tile_pool(name="osb", bufs=4))
        small = ctx.enter_context(tc.tile_pool(name="small", bufs=4))
        pwork = ctx.enter_context(tc.tile_pool(name="pwork", bufs=3, space="PSUM"))
        pout = ctx.enter_context(tc.tile_pool(name="pout", bufs=1, space="PSUM"))

        # ---- constants ----
        # Packed projection weights per d-chunk: [Wq | Wk] -> [128, dc, 128]
        w_qk = const.tile([P, ND, P], BF16)
        w_v = const.tile([P, ND, H], BF16)

        def load_weights():
            for w_dram, dst in ((wq_d, w_qk[:, :, 0:H]),
                                (wk_d, w_qk[:, :, H:P]),
                                (wv_d, w_v[:, :, :])):
                stg = wstage.tile([P, ND, H], FP32, tag="wstage",
                                  name=f"stg_{w_dram.tensor.name}")
                nc.gpsimd.dma_start(
                    out=stg[:],
                    in_=w_dram.rearrange("(dc p) h -> p dc h", p=P))
                nc.gpsimd.tensor_copy(out=dst, in_=stg[:])

        # f32 identity for the PE output transpose
        ident = const.tile([P, P], FP32)
        from concourse.masks import make_identity
        make_identity(nc, ident[:])
        ident_bf = const.tile([P, P], BF16)
        make_identity(nc, ident_bf[:])

        # 0/1 upper-triangular (incl. diagonal) bf16 mask in [tk, tq]
        # orientation: valid when tq >= tk  (col >= row).
        tri01 = const.tile([P, P], BF16)
        nc.gpsimd.memset(tri01[:], 1.0)
        nc.gpsimd.affine_select(
            out=tri01[:], in_=tri01[:],
            compare_op=mybir.AluOpType.is_ge,
            fill=0.0, base=0,
            pattern=[[1, P]], channel_multiplier=-1)

        # V_aug per chunk: [128 t-part, 4 k-tiles, 80] with col 64 = 1.0.
        # The k-tile stride is padded 65 -> 80 elements so each xbar
        # transpose writes at a 32-byte-aligned SBUF offset (the ucode
        # DMA-transpose silently corrupts on misaligned outputs).
        VA = 80
        v_aug = []
        for c in range(NC):
            va = vsb.tile([P, KPC, VA], BF16, tag=f"vaug{c}", name=f"vaug{c}")
            nc.gpsimd.memset(va[:, :, H:H + 1], 1.0)
            v_aug.append(va)

        # ---- x: per-chunk load-group -> cast -> xbar transpose zipper ----
        # Loads for chunk c and the transposes for chunk c alternate on the
        # SP queue so the DMA engines stream densely and chunk 0's x^T is
        # ready early.
        xt_chunks = [xtp.tile([P, ND, CW], BF16, tag=f"xT{c}", name=f"xT{c}")
                     for c in range(NC)]

        x_r = x_d.rearrange("(c a p) d -> c p a d", p=P, a=KPC)

        xfs = {}

        def load_x(c):
            xf = xload.tile([P, KPC, D], FP32, tag="xf", name=f"xf{c}")
            nc.sync.dma_start(out=xf[:], in_=x_r[c])
            xfs[c] = xf

        def cast_transpose(c, after=None):
            cast_eng = nc.vector
            for a in range(KPC):
                xb = xbf.tile([P, D], BF16, tag="xb", name=f"xb{c}_{a}")
                cast_eng.tensor_copy(out=xb[:], in_=xfs[c][:, a, :])
                if c == NC - 1:
                    # last chunk: transpose on the (idle) PE instead of the
                    # backlogged DMA xbar
                    ps_x = pwork.tile([P, ND, P], BF16, tag="pwork",
                                      name=f"ps_x{c}_{a}")
                    for dc in range(ND):
                        ti = nc.tensor.transpose(ps_x[:, dc, :],
                                                 xb[:, dc * P:(dc + 1) * P],
                                                 ident_bf[:])
                        if after is not None:
                            # keep the scheduler from hoisting these ahead
                            # of the previous block's PV matmuls (they wait
                            # on the last x load; the PVs do not)
                            tile.add_dep_helper(
                                ti.ins, after.ins, sync=False,
                                reason="x-transposes after block-A PVs")
                    nc.vector.tensor_copy(
                        out=xt_chunks[c][:, :, a * P:(a + 1) * P],
                        in_=ps_x[:])
                else:
                    nc.sync.dma_start(
                        out=xt_chunks[c][:, :, a * P:(a + 1) * P],
                        in_=xb[:],
                        transpose=True)

        load_x(0)
        load_weights()
        cast_transpose(0)
        load_x(1)
        cast_transpose(1)
        load_x(3)
        load_x(2)

        # ---- processing slots: chunks handled in order 0, 1, 3, 2 ----
        # x loads in the same order. Chunk 3's attention block (q-rows
        # [1536, 2048)) needs only qt(3) plus the early K/V chunks at its
        # head, so it starts the moment block A's exp stream drains; its
        # k-tiles run in the order [0-7, 12-15, 8-11] so the late-loaded
        # chunk 2's K/V never stall the ACT stream. Chunk 2's own block
        # (q-rows [1024, 1536)) runs last with everything already on-chip.
        qt_chunks, kt_chunks = {}, {}
        stores = []
        out_stage = []

        def proj_qk(c):
            xt = xt_chunks[c]
            ps_qk = pwork.tile([P, CW], FP32, tag="pwork", name=f"ps_qk{c}")
            for dc in range(ND):
                nc.tensor.matmul(ps_qk[:], w_qk[:, dc, :], xt[:, dc, :],
                                 start=(dc == 0), stop=(dc == ND - 1))
            qt = qkv.tile([H, CW], BF16, tag=f"qt{c}", name=f"qt{c}")
            kt = qkv.tile([H, CW], BF16, tag=f"kt{c}", name=f"kt{c}")
            nc.vector.tensor_copy(out=qt[:], in_=ps_qk[0:H, :])
            nc.vector.tensor_copy(out=kt[:], in_=ps_qk[H:P, :])
            qt_chunks[c] = qt
            kt_chunks[c] = kt

        def proj_v(c):
            # V projection; deferred past the first score matmuls of the
            # consuming attention block (V_aug is only read by that
            # block's last PV matmuls).
            xt = xt_chunks[c]
            ps_v = pwork.tile([H, CW], FP32, tag="pwork", name=f"ps_v{c}")
            for dc in range(ND):
                nc.tensor.matmul(ps_v[:], w_v[:, dc, :], xt[:, dc, :],
                                 start=(dc == 0), stop=(dc == ND - 1))
            vt = qkv.tile([H, CW], BF16, tag=f"vt{c}", name=f"vt{c}")
            nc.scalar.copy(out=vt[:], in_=ps_v[:])
            # V^T chunk -> V_aug k-tiles via one xbar transpose
            nc.sync.dma_start(out=v_aug[c][:, :, 0:H], in_=vt[:],
                              transpose=True)

        def attention(bc, qlo, aw, seq, inject, last_block=False):
            """Attention for q-rows [qlo, qlo+aw), k-tiles in `seq` order.
            `inject[idx]` = callables emitted at that sequence position
            (PE filler while ACT grinds exps). bc tags tile names.
            Returns the block's last PV instruction (an ordering anchor)."""
            ps_o = pout.tile([H + 1, aw], FP32, tag="pout", name=f"ps_o{bc}")
            pv_instrs = []

            def emit_s(ki):
                c0, j0 = ki // KPC, ki % KPC
                w = max(0, ki * P - qlo)
                ps_s = pwork.tile([P, aw], FP32, tag="pwork",
                                  name=f"ps_s{bc}_{ki}")
                kts = kt_chunks[c0][:, j0 * P:(j0 + 1) * P]
                for cq in range(qlo // CW, (qlo + aw) // CW):
                    lo = cq * CW - qlo       # block-local
                    hi = lo + CW
                    if hi <= w:
                        continue
                    s0 = max(w, lo)
                    nc.tensor.matmul(
                        ps_s[:, s0:hi], kts,
                        qt_chunks[cq][:, s0 - lo:CW],
                        start=True, stop=True)
                pt = ptp.tile([P, aw], BF16, tag="pt", name=f"pt{bc}_{ki}")
                nc.scalar.activation(
                    out=pt[:, w:aw], in_=ps_s[:, w:aw],
                    func=mybir.ActivationFunctionType.Exp,
                    scale=0.125)
                if ki * P >= qlo:
                    # causal diagonal: zero the strictly-lower triangle
                    nc.vector.tensor_mul(pt[:, w:w + P], pt[:, w:w + P],
                                         tri01[:])
                return pt, w

            def emit_pv(idx, ki, pt_w):
                pt, w = pt_w
                c0, j0 = ki // KPC, ki % KPC
                for cq in range(qlo // CW, (qlo + aw) // CW):
                    lo = cq * CW - qlo
                    hi = lo + CW
                    if hi <= w:
                        continue
                    s0 = max(w, lo)
                    pv_instrs.append(nc.tensor.matmul(
                        ps_o[:, s0:hi], v_aug[c0][:, j0, 0:H + 1],
                        pt[:, s0:hi],
                        start=(idx == 0), stop=(idx == len(seq) - 1)))

            def out_half(half):
                oth = otp.tile([H + 1, CW], FP32, tag="ot",
                               name=f"ot{bc}_{half}")
                nc.vector.tensor_copy(
                    out=oth[:], in_=ps_o[:, half * CW:(half + 1) * CW])
                pst = pwork.tile([P, KPC, H + 1], FP32, tag="pwork",
                                 name=f"psth{bc}_{half}")
                for j in range(KPC):
                    nc.tensor.transpose(pst[:, j, :],
                                        oth[:, j * P:(j + 1) * P],
                                        ident[0:H + 1, 0:H + 1])
                rec = small.tile([P, KPC], FP32, tag="rec",
                                 name=f"rech{bc}_{half}")
                nc.vector.reciprocal(rec[:], pst[:, :, H])
                ob = osb.tile([P, KPC, H], FP32, tag="ob",
                              name=f"obh{bc}_{half}")
                for j in range(KPC):
                    nc.vector.tensor_scalar_mul(
                        ob[:, j, :], pst[:, j, 0:H], rec[:, j:j + 1])
                stores.append(
                    (out_d.rearrange("(c a p) h -> c p a h",
                                     p=P, a=KPC)[qlo // CW + half], ob))

            pending = emit_s(seq[0])
            for idx, ki in enumerate(seq):
                nxt = emit_s(seq[idx + 1]) if idx + 1 < len(seq) else None
                for fn in inject.get(idx, ()):
                    fn()
                if idx in (5, 11) and out_stage:
                    out_stage.pop(0)()
                emit_pv(idx, ki, pending)
                pending = nxt

            # output stage: copy out of PSUM inline (frees the ps_o slot);
            # the PE transpose + rescale half(s) are deferred into the next
            # block unless this is the last one.
            nhalf = aw // CW
            if last_block:
                for half in range(nhalf):
                    out_half(half)
            else:
                # inline the PSUM copy by folding it into out_half, which
                # reads ps_o directly; defer the whole half stage.
                done = []

                for half in range(nhalf):
                    out_stage.append(lambda half=half: out_half(half))
            return pv_instrs[-1] if pv_instrs else None

        # slot 0: chunk 0 (no attention; V needed by block A's first PV)
        proj_qk(0)
        proj_v(0)

        # slot 1: chunk 1 + block A (q [0, 1024))
        proj_qk(1)
        a_anchor = attention(1, 0, 2 * CW, [0, 1, 7, 6, 2, 3, 5, 4],
                             {1: [lambda: proj_v(1)]})
        cast_transpose(3)

        # slot 2: chunk 3 + block B (q [1536, 2048)); chunk 2's cast/
        # transpose/projections are injected as PE filler mid-block
        proj_qk(3)
        attention(3, 3 * CW, CW,
                  list(range(8)) + list(range(12, 16)) + list(range(8, 12)),
                  {0: [lambda: cast_transpose(2)],
                   2: [lambda: proj_v(3)],
                   3: [lambda: proj_qk(2)],
                   4: [lambda: proj_v(2)]})

        # slot 3: chunk 2 + block C (q [1024, 1536))
        attention(2, 2 * CW, CW, list(range(8, 12)) + list(range(8)), {}, last_block=True)

        # Stores issue on SP last so they never block the transpose queue;
        # data dependencies still gate each store.
        for dst, ob in stores:
            nc.gpsimd.dma_start(out=dst, in_=ob[:])

        if dbg:
            dpool = ctx.enter_context(tc.tile_pool(name="dbg", bufs=1))
            d1 = dpool.tile([P, ND, CW], FP32, name="d1")
            nc.vector.tensor_copy(out=d1[:], in_=xt_chunks[0][:])
            nc.sync.dma_start(out=dbg["xt0"], in_=d1[:])
            d2 = dpool.tile([P, KPC, H + 1], FP32, name="d2")
            nc.vector.tensor_copy(out=d2[:], in_=v_aug[0][:, :, 0:H + 1])
            nc.sync.dma_start(out=dbg["vaug0"], in_=d2[:])
            d3 = dpool.tile([H, CW], FP32, name="d3")
            nc.vector.tensor_copy(out=d3[:], in_=qt_chunks[0][:])
            nc.sync.dma_start(out=dbg["qt0"], in_=d3[:])
            d4 = dpool.tile([H, CW], FP32, name="d4")
            nc.vector.tensor_copy(out=d4[:], in_=kt_chunks[0][:])
            nc.sync.dma_start(out=dbg["kt0"], in_=d4[:])


def _run(inputs, trace=False, **kw):
    global _compiled
    if _compiled is None:
        _compiled = _build()
    nc = _compiled
    x = np.ascontiguousarray(inputs["x"], dtype=np.float32)
    wq = np.ascontiguousarray(inputs["Wq"], dtype=np.float32)
    wk = np.ascontiguousarray(inputs["Wk"], dtype=np.float32)
    wv = np.ascontiguousarray(inputs["Wv"], dtype=np.float32)
    in_maps = [
        {"x": np.ascontiguousarray(x[i]), "Wq": wq, "Wk": wk, "Wv": wv}
        for i in range(B)
    ]
    res = run_bass_kernel_spmd(nc, in_maps, core_ids=list(range(B)),
                               trace=trace, **kw)
    out = np.stack([res.results[i]["out"] for i in range(B)], axis=0)
    return out, res


def kernel(x, Wq, Wk, Wv):
    out, _ = _run({"x": x, "Wq": Wq, "Wk": Wk, "Wv": Wv})
    return out

